# revision 6
# baseline (speedup 1.0000x reference)
"""GCN AutoEncoder (6-layer, BN+ReLU) on 8 Trainium2 NeuronCores.

v3 (on top of v2's dma_gather approach):
  - S selection matrices precomputed on host, streamed from DRAM on the idle
    HWDGE path (removes per-tile is_equal DVE builds)
  - dis_dst folded into S columns; self-loops folded in as ordinary edges
    with S value dis (removes PE transposes + DVE epilogue adds/mults)
  - PSUM->SBUF dis-scaling of the transform moved to the ACT engine
  - smaller gather groups + mg bufs=4 so all 4 SWDGE queues generate
    descriptors concurrently
"""
import os
import sys

sys.path.insert(0, "/opt/trn_rl_repo")

import numpy as np
import ml_dtypes

import concourse.bass as bass
import concourse.mybir as mybir
import concourse.tile as tile
from concourse import bacc
from concourse.bass_utils import run_bass_kernel_spmd

F32 = mybir.dt.float32
BF16 = mybir.dt.bfloat16
I16 = mybir.dt.int16
AF = mybir.ActivationFunctionType
ALU = mybir.AluOpType

NCORES = 8
P = 128
ROWW = 128          # table row width (bf16 elems) -> 256B, dma_gather granule
LO_ROWS = 32768     # int16 idx limit: rows below go in the "lo" gather
GBLK = 48           # max gathered chunks per group (48*256B = 12KB/partition)


class Cfg:
    def __init__(self, n_nodes=50000, dims=None):
        self.n = n_nodes
        self.dims = dims or [(88, 70), (70, 60), (60, 50), (50, 60), (60, 70), (70, 88)]
        self.relu = [True, True, False, True, True, False]
        self.bn = [True, True, False, True, True, False]
        self.npc = self.n // NCORES
        assert self.npc * NCORES == self.n
        self.ntiles = (self.npc + P - 1) // P
        self.m_last = self.npc - (self.ntiles - 1) * P
        self.eps = 1e-5


def preprocess(cfg, x, edge_index):
    """Host-side: degrees/dis, per-(tile, lo/hi) chunked edge idx arrays laid
    out for dma_gather, host-built S matrices (dis_dst folded into columns,
    self-loops as ordinary edges), and node-major transform aux."""
    n, npc, ntiles = cfg.n, cfg.npc, cfg.ntiles
    src0 = np.asarray(edge_index[0], dtype=np.int64).astype(np.int32)
    dst0 = np.asarray(edge_index[1], dtype=np.int64).astype(np.int32)
    deg = np.bincount(dst0, minlength=n).astype(np.float32) + 1.0  # + self loop
    dis = 1.0 / np.sqrt(deg)
    # self-loops folded in as ordinary edges (S value = dis handles the norm)
    loop = np.arange(n, dtype=np.int32)
    src = np.concatenate([src0, loop])
    dst = np.concatenate([dst0, loop])

    trows = ntiles * P  # table rows per rank block
    r = (src // npc) * trows + (src % npc)  # row in AllGather table
    core_of = dst // npc
    dloc = dst % npc
    tile_of = dloc // P
    drel_of = dloc - tile_of * P
    half_of = (r >= LO_ROWS).astype(np.int64)
    disd = dis[dst]

    # per (core, tile, half) counts -> equalized chunk counts
    counts = np.zeros((NCORES, ntiles, 2), dtype=np.int64)
    np.add.at(counts, (core_of, tile_of, half_of), 1)
    assert counts.min() > 0, "empty (core,tile,half) bucket; padding logic assumes >0"
    cts = np.ceil(counts.max(axis=0) / P).astype(np.int64)  # [ntiles, 2]

    # groups of tiles, bounded by GBLK chunks
    groups = []
    cur, cur_blk = [], 0
    for t in range(ntiles):
        tb = int(cts[t, 0] + cts[t, 1])
        if cur and cur_blk + tb > GBLK:
            groups.append(cur)
            cur, cur_blk = [], 0
        cur.append(t)
        cur_blk += tb
    if cur:
        groups.append(cur)

    # chunk layout:
    #  - gather order (idx array): per group: [lo chunks of tiles in g][hi chunks]
    #  - S cols (per-tile stream): per tile contiguous: [lo chunks][hi chunks]
    tile_blk = {}       # t -> list of group-relative block indices (lo then hi)
    tile_c0 = {}        # t -> first S chunk-col
    group_meta = []     # per group: (tiles, nblk_lo, nblk_hi, idx_q0)
    idx_pos = {}        # (t, h) -> flat idx start position
    q = 0
    for g in groups:
        nblk_lo = int(sum(cts[t, 0] for t in g))
        nblk_hi = int(sum(cts[t, 1] for t in g))
        group_meta.append((g, nblk_lo, nblk_hi, q))
        off = 0
        for t in g:
            idx_pos[(t, 0)] = q + off * P
            tile_blk[t] = list(range(off, off + int(cts[t, 0])))
            off += int(cts[t, 0])
        for t in g:
            idx_pos[(t, 1)] = q + off * P
            tile_blk[t] += list(range(off, off + int(cts[t, 1])))
            off += int(cts[t, 1])
        q += (nblk_lo + nblk_hi) * P
    tot_idxs = q
    tot_chunks = tot_idxs // P

    c0 = 0
    for t in range(ntiles):
        tile_c0[t] = c0
        c0 += int(cts[t, 0] + cts[t, 1])

    idx_flat = np.zeros((NCORES, tot_idxs), dtype=np.int16)
    s_host = np.zeros((NCORES, P, tot_chunks * P), dtype=ml_dtypes.bfloat16)

    for i in range(NCORES):
        m = core_of == i
        ri, ti, hi_, di, dd = r[m], tile_of[m], half_of[m], drel_of[m], disd[m]
        order = np.lexsort((ri, hi_, ti))
        ri, ti, hi_, di, dd = ri[order], ti[order], hi_[order], di[order], dd[order]
        key = ti * 2 + hi_
        first = np.r_[True, key[1:] != key[:-1]]
        gstart = np.flatnonzero(first)
        pos = np.arange(len(key)) - np.repeat(
            gstart, np.diff(np.r_[gstart, len(key)]))
        # idx array position (gather order)
        base = np.array([idx_pos[(t, h)] for t, h in zip(ti[first], hi_[first])])
        flatp = np.repeat(base, np.diff(np.r_[gstart, len(key)])) + pos
        idx_flat[i, flatp] = (ri - hi_ * LO_ROWS).astype(np.int16)
        # S position (per-tile order): col = (c0[t] + (h? ct_lo:0) + pos//P)*P + drel
        sc0 = np.array([tile_c0[t] + (int(cts[t, 0]) if h else 0)
                        for t, h in zip(ti[first], hi_[first])])
        scol = (np.repeat(sc0, np.diff(np.r_[gstart, len(key)])) + pos // P) * P + di
        s_host[i, pos % P, scol] = dd.astype(ml_dtypes.bfloat16)

    # idx SBUF wrap: flat j -> partition j%16 (replicated x8), col j//16
    idx_cols = tot_idxs // 16
    idx_sb = np.zeros((NCORES, P, idx_cols), dtype=np.int16)
    for i in range(NCORES):
        w = idx_flat[i].reshape(idx_cols, 16).T  # [16, cols]
        idx_sb[i] = np.tile(w, (8, 1))

    # per-core node-major aux
    xs = np.asarray(x, dtype=np.float32)
    f_in0 = xs.shape[1]
    xT = np.zeros((NCORES, f_in0, trows), dtype=ml_dtypes.bfloat16)
    dis_col = np.zeros((NCORES, P, ntiles), dtype=np.float32)
    for i in range(NCORES):
        sl = slice(i * npc, (i + 1) * npc)
        xT[i, :, :npc] = xs[sl].T.astype(ml_dtypes.bfloat16)
        d = dis[sl]
        dis_col[i, : npc - (ntiles - 1) * P, ntiles - 1] = d[(ntiles - 1) * P:]
        for t in range(ntiles - 1):
            dis_col[i, :, t] = d[t * P:(t + 1) * P]

    return dict(
        idx_sb=idx_sb, s_host=s_host, cts=cts, groups=groups,
        group_meta=group_meta, tile_blk=tile_blk, tile_c0=tile_c0,
        tot_chunks=tot_chunks, tot_idxs=tot_idxs,
        xT=xT, dis_col=dis_col,
    )


def build_nc(cfg, pre):
    n, npc, ntiles, m_last = cfg.n, cfg.npc, cfg.ntiles, cfg.m_last
    dims = cfg.dims
    cts = pre["cts"]
    group_meta = pre["group_meta"]
    tile_blk = pre["tile_blk"]
    tile_c0 = pre["tile_c0"]
    tot_chunks = pre["tot_chunks"]
    tot_idxs = pre["tot_idxs"]
    trows = ntiles * P
    fmax = max(fo for _, fo in dims)
    f_in0 = dims[0][0]
    rg = [list(range(NCORES))]
    idx_cols = tot_idxs // 16

    nc = bacc.Bacc("TRN2", target_bir_lowering=False, debug=False,
                   num_devices=NCORES, num_swdge_queues=4)

    # ---- external IO
    xT_e = nc.dram_tensor("xT", [f_in0, trows], BF16, kind="ExternalInput")
    idx_e = nc.dram_tensor("idx", [P, idx_cols], I16, kind="ExternalInput")
    s_e = nc.dram_tensor("smat", [P, tot_chunks * P], BF16, kind="ExternalInput")
    dis_col_e = nc.dram_tensor("dis_col", [P, ntiles], F32, kind="ExternalInput")
    b6_rep_e = nc.dram_tensor("b6_rep", [P, dims[5][1]], F32, kind="ExternalInput")
    w_e, b_e, g_e, be_e = [], [], [], []
    for l, (fi, fo) in enumerate(dims):
        w_e.append(nc.dram_tensor(f"W{l}", [fi, fo], BF16, kind="ExternalInput"))
        b_e.append(nc.dram_tensor(f"b{l}", [fo, 1], F32, kind="ExternalInput"))
        if cfg.bn[l]:
            g_e.append(nc.dram_tensor(f"g{l}", [fo, 1], F32, kind="ExternalInput"))
            be_e.append(nc.dram_tensor(f"be{l}", [fo, 1], F32, kind="ExternalInput"))
        else:
            g_e.append(None)
            be_e.append(None)
    out_e = nc.dram_tensor("out", [trows, dims[5][1]], F32, kind="ExternalOutput")

    with tile.TileContext(nc) as tc:
        with (
            tc.tile_pool(name="const", bufs=1) as cpool,
            tc.tile_pool(name="vt", bufs=2) as vtpool,
            tc.tile_pool(name="tsb", bufs=2) as tpool,
            tc.tile_pool(name="mg", bufs=4) as mpool,
            tc.tile_pool(name="ssb", bufs=4) as spool,
            tc.tile_pool(name="eps", bufs=4) as epool,
            tc.tile_pool(name="stat", bufs=2) as stpool,
            tc.tile_pool(name="psA", bufs=3, space="PSUM") as psA,
            tc.tile_pool(name="psB", bufs=2, space="PSUM") as psB,
            tc.tile_pool(name="dram", bufs=1, space="DRAM") as dram,
        ):
            # ---- load constants to SBUF
            def load(pool, e, shape, dtype=F32):
                t = pool.tile(shape, dtype, name=f"c_{e.name}")
                nc.sync.dma_start(t[:], e[:])
                return t

            xT_sb = load(cpool, xT_e, [f_in0, trows], BF16)
            idx_sb = load(cpool, idx_e, [P, idx_cols], I16)
            dcol_sb = load(cpool, dis_col_e, [P, ntiles])
            b6r_sb = load(cpool, b6_rep_e, [P, dims[5][1]])
            w_sb = [load(cpool, w_e[l], [dims[l][0], dims[l][1]], BF16) for l in range(6)]
            b_sb = [load(cpool, b_e[l], [dims[l][1], 1]) for l in range(6)]
            g_sb = [load(cpool, g_e[l], [dims[l][1], 1]) if cfg.bn[l] else None for l in range(6)]
            be_sb = [load(cpool, be_e[l], [dims[l][1], 1]) if cfg.bn[l] else None for l in range(6)]

            # DRAM comm buffers (table rows padded to ROWW bf16 = 256B)
            ag_in = [dram.tile([trows, ROWW], BF16, tag=f"agin{l}", name=f"agin{l}") for l in range(6)]
            ag_out = [dram.tile([NCORES * trows, ROWW], BF16, tag=f"agout{l}", name=f"agout{l}", addr_space="Shared") for l in range(6)]
            ar_in = [dram.tile([dims[l][1], 2], F32, tag=f"arin{l}", name=f"arin{l}") if cfg.bn[l] else None for l in range(6)]
            ar_out = [dram.tile([dims[l][1], 2], F32, tag=f"arout{l}", name=f"arout{l}", addr_space="Shared") if cfg.bn[l] else None for l in range(6)]

            prev_vT = None       # [F_in, trows] bf16 post-activation (pre-bn)
            bn_cur = None        # (gs, cv) per-partition affine for pending bn

            for l in range(6):
                f_in, f_out = dims[l]
                tile_ms = [P] * (ntiles - 1) + [m_last]

                # ---------- transform: t = dis * (bn(v) @ W)  [node-major bf16]
                t_sb = tpool.tile([P, ntiles * ROWW], BF16, tag="tsb", name="tsb")
                for t in range(ntiles):
                    m = tile_ms[t]
                    lhsT = (xT_sb if l == 0 else prev_vT)[:f_in, t * P:t * P + m]
                    if bn_cur is not None:
                        gs_c, cv_c = bn_cur
                        vbn = epool.tile([fmax, P], BF16, tag="vbn", name="vbn")
                        nc.scalar.activation(vbn[:f_in, :m], lhsT, AF.Identity,
                                             bias=cv_c[:f_in, 0:1],
                                             scale=gs_c[:f_in, 0:1])
                        lhsT = vbn[:f_in, :m]
                    tps = psB.tile([P, f_out], F32, tag="tps", name="tps")
                    nc.tensor.matmul(tps[:m, :], lhsT=lhsT, rhs=w_sb[l][:f_in, :f_out],
                                     start=True, stop=True)
                    tsl = t_sb[:m, t * ROWW:t * ROWW + f_out]
                    # dis scaling + f32->bf16 cast on the ACT engine (reads PSUM)
                    nc.scalar.activation(tsl, tps[:m, :], AF.Identity,
                                         scale=dcol_sb[:m, t:t + 1])
                nc.sync.dma_start(
                    ag_in[l][:].rearrange("(t p) f -> p t f", p=P),
                    t_sb[:].rearrange("p (t f) -> p t f", f=ROWW))

                # ---------- AllGather
                nc.gpsimd.collective_compute(
                    "AllGather", ALU.bypass,
                    ins=[ag_in[l][:].opt()],
                    outs=[ag_out[l][:].opt()],
                    replica_groups=rg,
                )

                # ---------- aggregation
                if cfg.bn[l]:
                    ssum = stpool.tile([f_out, ntiles], F32, tag="ssum", name="ssum")
                    ssq = stpool.tile([f_out, ntiles], F32, tag="ssq", name="ssq")
                if l < 5:
                    vT = vtpool.tile([fmax, trows], BF16, tag="vt", name="vt")

                for gi, (gtiles, nblk_lo, nblk_hi, idx_q0) in enumerate(group_meta):
                    nblk = nblk_lo + nblk_hi
                    mg = mpool.tile([P, nblk * ROWW], BF16, tag="mg", name="mg")
                    mg3 = mg[:].rearrange("p (b f) -> p b f", f=ROWW)
                    ic0 = idx_q0 // 16
                    if nblk_lo:
                        nc.gpsimd.dma_gather(
                            mg3[:, 0:nblk_lo, :], ag_out[l][0:LO_ROWS, :],
                            idx_sb[:, ic0:ic0 + nblk_lo * 8],
                            nblk_lo * P, nblk_lo * P, ROWW,
                            queue_num=(2 * gi) % 4, single_packet=False)
                    if nblk_hi:
                        nc.gpsimd.dma_gather(
                            mg3[:, nblk_lo:nblk, :],
                            ag_out[l][LO_ROWS:NCORES * trows, :],
                            idx_sb[:, ic0 + nblk_lo * 8:ic0 + nblk * 8],
                            nblk_hi * P, nblk_hi * P, ROWW,
                            queue_num=(2 * gi + 1) % 4, single_packet=False)

                    for t in gtiles:
                        m = tile_ms[t]
                        ct = int(cts[t, 0] + cts[t, 1])
                        # S matrices streamed from DRAM (dis_dst + self-loop folded)
                        s = spool.tile([P, ct * P], BF16, tag="ssb", name="ssb")
                        nc.sync.dma_start(
                            s[:], s_e[:, tile_c0[t] * P:(tile_c0[t] + ct) * P])
                        if l < 5:
                            agg = psA.tile([f_out, P], F32, tag="agg", name="agg")
                        else:
                            agg = psA.tile([P, f_out], F32, tag="agg", name="agg")
                        for j, blk in enumerate(tile_blk[t]):
                            lhsT = mg[:, blk * ROWW:blk * ROWW + f_out]
                            ssl = s[:, j * P:j * P + m]
                            if l < 5:
                                nc.tensor.matmul(agg[:f_out, :m], lhsT=lhsT,
                                                 rhs=ssl, start=(j == 0),
                                                 stop=(j == ct - 1))
                            else:
                                nc.tensor.matmul(agg[:m, :f_out], lhsT=ssl,
                                                 rhs=lhsT, start=(j == 0),
                                                 stop=(j == ct - 1))

                        if l < 5:
                            # epilogue: v = act(agg + b) straight from PSUM
                            vsl = vT[:f_out, t * P:t * P + m]
                            nc.scalar.activation(
                                vsl, agg[:f_out, :m],
                                AF.Relu if cfg.relu[l] else AF.Identity,
                                bias=b_sb[l][:f_out, 0:1])
                            if cfg.bn[l]:
                                nc.vector.tensor_reduce(
                                    out=ssum[:f_out, t:t + 1], in_=vsl,
                                    axis=mybir.AxisListType.X, op=ALU.add)
                                sq = epool.tile([fmax, P], F32, tag="esq", name="esq")
                                nc.vector.tensor_tensor(out=sq[:f_out, :m], in0=vsl,
                                                        in1=vsl, op=ALU.mult)
                                nc.vector.tensor_reduce(
                                    out=ssq[:f_out, t:t + 1], in_=sq[:f_out, :m],
                                    axis=mybir.AxisListType.X, op=ALU.add)
                        else:
                            osl = epool.tile([P, f_out], F32, tag="osl", name="osl")
                            nc.vector.tensor_tensor(out=osl[:m, :], in0=agg[:m, :f_out],
                                                    in1=b6r_sb[:m, :f_out], op=ALU.add)
                            nc.sync.dma_start(out_e[t * P:t * P + m, :], osl[:m, :])

                # ---------- stats AllReduce + fold into next-layer affine
                if l < 5:
                    if cfg.bn[l]:
                        pack = stpool.tile([f_out, 2], F32, tag="pack", name="pack")
                        nc.vector.tensor_reduce(out=pack[:f_out, 0:1],
                                                in_=ssum[:f_out, :ntiles],
                                                axis=mybir.AxisListType.X, op=ALU.add)
                        nc.vector.tensor_reduce(out=pack[:f_out, 1:2],
                                                in_=ssq[:f_out, :ntiles],
                                                axis=mybir.AxisListType.X, op=ALU.add)
                        nc.sync.dma_start(ar_in[l][:], pack[:f_out, :])
                        nc.gpsimd.collective_compute(
                            "AllReduce", ALU.add,
                            ins=[ar_in[l][:].opt()],
                            outs=[ar_out[l][:].opt()],
                            replica_groups=rg,
                        )
                        st = stpool.tile([f_out, 2], F32, tag="st", name="st")
                        nc.sync.dma_start(st[:f_out, :], ar_out[l][:])
                        mu = stpool.tile([f_out, 1], F32, tag="mu", name="mu")
                        nc.vector.tensor_scalar_mul(mu[:f_out, :], st[:f_out, 0:1], 1.0 / cfg.n)
                        msq = stpool.tile([f_out, 1], F32, tag="msq", name="msq")
                        nc.vector.tensor_scalar_mul(msq[:f_out, :], st[:f_out, 1:2], 1.0 / cfg.n)
                        var = stpool.tile([f_out, 1], F32, tag="var", name="var")
                        nc.vector.tensor_tensor(out=var[:f_out, :], in0=mu[:f_out, :],
                                                in1=mu[:f_out, :], op=ALU.mult)
                        nc.vector.tensor_tensor(out=var[:f_out, :], in0=msq[:f_out, :],
                                                in1=var[:f_out, :], op=ALU.subtract)
                        nc.vector.tensor_scalar_add(var[:f_out, :], var[:f_out, :], cfg.eps)
                        rv = stpool.tile([f_out, 1], F32, tag="rv", name="rv")
                        nc.vector.reciprocal(rv[:f_out, :], var[:f_out, :])
                        rstd = stpool.tile([f_out, 1], F32, tag="rstd", name="rstd")
                        nc.scalar.activation(rstd[:f_out, :], rv[:f_out, :], AF.Sqrt)
                        gs = stpool.tile([f_out, 1], F32, tag="gs", name="gs")
                        nc.vector.tensor_tensor(out=gs[:f_out, :], in0=g_sb[l][:f_out, :],
                                                in1=rstd[:f_out, :], op=ALU.mult)
                        cv = stpool.tile([f_out, 1], F32, tag="cv", name="cv")
                        nc.vector.tensor_tensor(out=cv[:f_out, :], in0=gs[:f_out, :],
                                                in1=mu[:f_out, :], op=ALU.mult)
                        nc.vector.tensor_tensor(out=cv[:f_out, :], in0=be_sb[l][:f_out, :],
                                                in1=cv[:f_out, :], op=ALU.subtract)
                        bn_cur = (gs, cv)
                    else:
                        bn_cur = None
                    prev_vT = vT

    nc.compile()
    return nc


_CACHE = {}
LAST_RES = None


def _get_compiled(cfg, key, pre):
    if key not in _CACHE:
        _CACHE[key] = build_nc(cfg, pre)
    return _CACHE[key]


def _run(inputs, trace=False):
    cfg = Cfg(n_nodes=int(np.asarray(inputs["x"]).shape[0]))
    x = np.asarray(inputs["x"], dtype=np.float32)
    edge_index = np.asarray(inputs["edge_index"])
    pre = preprocess(cfg, x, edge_index)
    key = (cfg.n, edge_index.shape[1], hash(edge_index.tobytes()))
    nc = _get_compiled(cfg, key, pre)

    b6_rep = np.tile(np.asarray(inputs["b6"], dtype=np.float32)[None, :], (P, 1))
    bn_map = {0: "1", 1: "2", 3: "3", 4: "4"}
    in_maps = []
    for i in range(NCORES):
        m = {
            "xT": pre["xT"][i],
            "idx": pre["idx_sb"][i],
            "smat": pre["s_host"][i],
            "dis_col": pre["dis_col"][i],
            "b6_rep": b6_rep,
        }
        for l in range(6):
            m[f"W{l}"] = np.asarray(inputs[f"W{l+1}"], dtype=np.float32).astype(ml_dtypes.bfloat16)
            m[f"b{l}"] = np.asarray(inputs[f"b{l+1}"], dtype=np.float32)[:, None]
            if cfg.bn[l]:
                m[f"g{l}"] = np.asarray(inputs[f"g{bn_map[l]}"], dtype=np.float32)[:, None]
                m[f"be{l}"] = np.asarray(inputs[f"be{bn_map[l]}"], dtype=np.float32)[:, None]
        in_maps.append(m)

    res = run_bass_kernel_spmd(nc, in_maps, core_ids=list(range(NCORES)), trace=trace)
    global LAST_RES
    LAST_RES = res
    parts = [res.results[i]["out"][:cfg.npc] for i in range(NCORES)]
    out = np.concatenate(parts, axis=0)
    return out, res.exec_time_ns


def kernel(**inputs) -> np.ndarray:
    out, _ = _run(inputs, trace=False)
    return out


def kernel_traced(**inputs):
    # NTFF profile hook is registered at interpreter boot (antenv.axon_hooks)
    return _run(inputs, trace=True)
